# revision 1
# baseline (speedup 1.0000x reference)
"""Multi-head self-attention on 8 TRN2 NeuronCores.

Strategy: tensor-parallel over heads. Each core owns 2 of the 16 heads
(a 128-column slice of Wq/Wk/Wv and the matching 128-row slice of Wo) and
computes, for both batch elements:
  - its QKV projection columns, laid out transposed [cols, tokens],
  - full attention for its (batch, head) pairs via transposed scores
    [k, q] so every matmul keeps a 512-wide moving free dim (full rate),
    with a ones-column appended to V so the softmax normalizer falls out of
    the same matmul,
  - a partial output projection against its Wo row-slice.
The host sums the 8 partial outputs (the tensor-parallel all-reduce) and
adds the output bias.

Precision: weights/activations matmuls run in float32r (fp32 truncated to
~FP22); the scores matmul runs in bf16 (its error is crushed by the 1/8
softmax scale); softmax runs in fp32 on the scalar engine. Scores here are
O(1) so exp needs no max-subtraction. The softmax normalizer reciprocal is
batched 4-rows-at-a-time and broadcast across partitions via a DRAM bounce.
"""

import numpy as np

B, S, D, H, DK = 2, 2048, 1024, 16, 64
NCORES = 8
HPC = H // NCORES          # heads per core = 2
COLS = HPC * DK            # feature columns per core = 128
T = B * S                  # 4096 tokens
TCH = 512                  # token chunk (max fp32 moving free dim)
NT = T // TCH              # 8
ND = D // 128              # 8 contraction chunks
NE = D // 128              # 8 output-column chunks
NQ = S // TCH              # 4 query chunks per batch
NKT = S // 128             # 16 key tiles per batch

_CACHE = {}


def _build_program():
    from contextlib import ExitStack

    import concourse.bacc as bacc
    import concourse.mybir as mybir
    import concourse.tile as tile
    from concourse.masks import make_identity

    f32 = mybir.dt.float32
    f32r = mybir.dt.float32r
    f16 = mybir.dt.float16
    Exp = mybir.ActivationFunctionType.Exp

    nc = bacc.Bacc("TRN2", target_bir_lowering=False, debug=False,
                   num_devices=NCORES)

    XT = nc.dram_tensor("xt", [D, T], f32, kind="ExternalInput").ap()
    Wq = nc.dram_tensor("wq", [D, COLS], f32, kind="ExternalInput").ap()
    Wk = nc.dram_tensor("wk", [D, COLS], f32, kind="ExternalInput").ap()
    Wv = nc.dram_tensor("wv", [D, COLS], f32, kind="ExternalInput").ap()
    Bq = nc.dram_tensor("bq", [COLS, 1], f32, kind="ExternalInput").ap()
    Bk = nc.dram_tensor("bk", [COLS, 1], f32, kind="ExternalInput").ap()
    Bv = nc.dram_tensor("bv", [COLS, 1], f32, kind="ExternalInput").ap()
    Wo = nc.dram_tensor("wo", [COLS, D], f32, kind="ExternalInput").ap()
    OT = nc.dram_tensor("ot", [D, T], f32, kind="ExternalOutput").ap()

    import concourse.bass as bass

    with tile.TileContext(nc) as tc, ExitStack() as ctx:
        consts = ctx.enter_context(tc.tile_pool(name="consts", bufs=1))
        xtp = ctx.enter_context(tc.tile_pool(name="xtp", bufs=16))
        big = ctx.enter_context(tc.tile_pool(name="big", bufs=1))
        vpp = ctx.enter_context(tc.tile_pool(name="vpp", bufs=1))
        expp = ctx.enter_context(tc.tile_pool(name="expp", bufs=4))
        attnp = ctx.enter_context(tc.tile_pool(name="attnp", bufs=8))
        osbp = ctx.enter_context(tc.tile_pool(name="osbp", bufs=4))
        zrp = ctx.enter_context(tc.tile_pool(name="zrp", bufs=2))
        zbp = ctx.enter_context(tc.tile_pool(name="zbp", bufs=4))
        drp = ctx.enter_context(tc.tile_pool(name="drp", bufs=2, space="DRAM"))
        ps_acc = ctx.enter_context(tc.tile_pool(name="ps_acc", bufs=4, space="PSUM"))
        ps_s = ctx.enter_context(tc.tile_pool(name="ps_s", bufs=2, space="PSUM"))
        ps_o = ctx.enter_context(tc.tile_pool(name="ps_o", bufs=2, space="PSUM"))

        # ---- constants needed by phase 1 first (shortens the PE cold start)
        w_sb = {}
        b_sb = {}
        for nm, src, bsrc in (("wq", Wq, Bq), ("wk", Wk, Bk), ("wv", Wv, Bv)):
            w_sb[nm] = []
            for d in range(ND):
                wt = consts.tile([128, COLS], f32r, name=f"{nm}_{d}")
                nc.sync.dma_start(wt, src[d * 128:(d + 1) * 128, :].bitcast(f32r))
                w_sb[nm].append(wt)
            bt = consts.tile([COLS, 1], f32, name=f"b{nm[1]}_sb")
            nc.sync.dma_start(bt, bsrc)
            b_sb[nm] = bt

        # ---- phase 1: QKV projections, outputs transposed [cols, tokens] ----
        QT = big.tile([COLS, T], f16, name="QT")
        KT = big.tile([COLS, T], f16, name="KT")
        VT = big.tile([COLS, T], f32, name="VT")
        proj_out = {"wq": QT, "wk": KT, "wv": VT}
        for t in range(NT):
            xts = []
            for d in range(ND):
                xt_t = xtp.tile([128, TCH], f32r, name=f"xt_{t}_{d}", tag="xt")
                nc.gpsimd.dma_start(
                    xt_t,
                    XT[d * 128:(d + 1) * 128, t * TCH:(t + 1) * TCH].bitcast(f32r))
                xts.append(xt_t)
            for nm in ("wq", "wk", "wv"):
                pacc = ps_acc.tile([COLS, TCH], f32, tag="acc",
                                   name=f"pacc_{t}_{nm}")
                for d in range(ND):
                    nc.tensor.matmul(pacc, lhsT=w_sb[nm][d], rhs=xts[d],
                                     start=(d == 0), stop=(d == ND - 1))
                nc.vector.tensor_scalar_add(
                    proj_out[nm][:, t * TCH:(t + 1) * TCH], pacc, b_sb[nm])

        # ---- remaining constants ----
        wo_sb = []
        for e in range(NE):
            wt = consts.tile([128, 128], f32r, name=f"wo_{e}")
            nc.sync.dma_start(wt, Wo[:, e * 128:(e + 1) * 128].bitcast(f32r))
            wo_sb.append(wt)
        ident = consts.tile([128, 128], f32, name="ident")
        make_identity(nc, ident)
        onecol = consts.tile([128, 1], f32, name="onecol")
        nc.vector.memset(onecol, 1.0)

        # ---- phase 2: build V' tiles [128 tokens, 65] per (b, h, ktile) ----
        vp_sb = [[[None] * NKT for _ in range(HPC)] for _ in range(B)]
        for kb in range(T // 128):
            b, kt = kb // NKT, kb % NKT
            ptr = ps_s.tile([128, 128], f32, tag="s", name=f"ptr_{kb}")
            nc.tensor.transpose(ptr, VT[:, kb * 128:(kb + 1) * 128], ident)
            for h in range(HPC):
                vp = vpp.tile([128, DK + 1], f16, name=f"vp_{b}_{h}_{kt}")
                nc.vector.tensor_copy(vp[:, 0:DK], ptr[:, h * DK:(h + 1) * DK])
                nc.vector.tensor_copy(vp[:, DK:DK + 1], onecol)
                vp_sb[b][h][kt] = vp

        # ---- phases 3+4 per batch: attention then partial output proj ----
        for b in range(B):
            attn_sb = [
                attnp.tile([128, TCH], f32r, tag="attn", name=f"attn_{b}_{q}")
                for q in range(NQ)
            ]
            for h in range(HPC):
                hs = slice(h * DK, (h + 1) * DK)
                # gather of the 4 softmax-normalizer rows at partitions
                # {0,32,64,96} for one batched reciprocal
                zg = zrp.tile([128, TCH], f32, tag="zg", name=f"zg_{b}_{h}")
                nc.vector.memset(zg, 1.0)
                araw = []
                for q in range(NQ):
                    qs = slice(b * S + q * TCH, b * S + (q + 1) * TCH)
                    patt = ps_acc.tile([DK + 1, TCH], f32, tag="acc",
                                       name=f"patt_{b}_{h}_{q}")
                    for kt in range(NKT):
                        ks = slice(b * S + kt * 128, b * S + (kt + 1) * 128)
                        sps = ps_s.tile([128, TCH], f32, tag="s",
                                        name=f"sc_{b}_{h}_{kt}_{q}")
                        nc.tensor.matmul(sps, lhsT=KT[hs, ks], rhs=QT[hs, qs],
                                         start=True, stop=True)
                        esb = expp.tile([128, TCH], f16, tag="exp",
                                        name=f"exp_{b}_{h}_{kt}_{q}")
                        nc.scalar.activation(esb, sps, Exp, scale=0.125)
                        nc.tensor.matmul(patt, lhsT=vp_sb[b][h][kt],
                                         rhs=esb, start=(kt == 0),
                                         stop=(kt == NKT - 1))
                    # drain the accumulator to SBUF promptly so the PSUM slot
                    # frees for the next q/head instead of waiting on the
                    # normalizer chain
                    ar = zbp.tile([DK, TCH], f32, tag="araw",
                                  name=f"araw_{b}_{h}_{q}")
                    nc.vector.tensor_copy(ar, patt[0:DK, :])
                    araw.append(ar)
                    nc.vector.tensor_copy(zg[32 * q:32 * q + 1, :],
                                          patt[DK:DK + 1, :])
                zgr = zrp.tile([128, TCH], f32, tag="zgr", name=f"zgr_{b}_{h}")
                nc.vector.reciprocal(zgr, zg)
                scratch = drp.tile([NQ, TCH], f32, tag="scr",
                                   name=f"scr_{b}_{h}")
                nc.gpsimd.dma_start(scratch, zgr[0:128:32, :])
                for q in range(NQ):
                    zbs = zbp.tile([DK, TCH], f32, tag="zbs",
                                   name=f"zbs_{b}_{h}_{q}")
                    row = scratch[q:q + 1, :]
                    nc.gpsimd.dma_start(
                        zbs,
                        bass.AP(tensor=row.tensor, offset=row.offset,
                                ap=[[0, DK]] + row.ap[1:]))
                    nc.vector.tensor_mul(attn_sb[q][hs, :], araw[q], zbs)
            for q in range(NQ):
                ts_g = slice(b * S + q * TCH, b * S + (q + 1) * TCH)
                for e in range(NE):
                    po = ps_o.tile([128, TCH], f32, tag="o",
                                   name=f"po_{b}_{q}_{e}")
                    nc.tensor.matmul(po, lhsT=wo_sb[e], rhs=attn_sb[q],
                                     start=True, stop=True)
                    ob = osbp.tile([128, TCH], f32, tag="osb",
                                   name=f"ob_{b}_{q}_{e}")
                    nc.vector.tensor_copy(ob, po)
                    nc.sync.dma_start(OT[e * 128:(e + 1) * 128, ts_g], ob)

    nc.compile()
    return nc


def _get_program():
    if "nc" not in _CACHE:
        _CACHE["nc"] = _build_program()
    return _CACHE["nc"]


def _install_ntff_hook():
    """Provide the antenv.axon_hooks shim this container's antenv lacks so
    run_bass_kernel_spmd(trace=True) can capture NTFF profiles."""
    import sys
    import types

    try:
        import antenv

        if hasattr(antenv, "axon_hooks"):
            return
        mod = types.ModuleType("antenv.axon_hooks")
        mod._hook = None
        mod.set_axon_ntff_profile_hook = lambda h: setattr(mod, "_hook", h)
        mod.get_axon_ntff_profile_hook = lambda: mod._hook
        sys.modules["antenv.axon_hooks"] = mod
        antenv.axon_hooks = mod
        from trn_agent_boot.trn_boot import _ntff_profile_via_ctypes

        mod.set_axon_ntff_profile_hook(
            _ntff_profile_via_ctypes("/opt/axon/libaxon_pjrt.so"))
    except Exception:
        pass


def kernel(X, Wq, bq, Wk, bk, Wv, bv, Wo, bo, _profile=False, _trace_cores=None):
    from concourse.bass_utils import run_bass_kernel_spmd

    if _profile:
        _install_ntff_hook()

    nc = _get_program()

    XT = np.ascontiguousarray(np.asarray(X, np.float32).reshape(T, D).T)
    Wq, Wk, Wv, Wo = (np.asarray(w, np.float32) for w in (Wq, Wk, Wv, Wo))
    bq, bk, bv, bo = (np.asarray(v, np.float32) for v in (bq, bk, bv, bo))

    in_maps = []
    for c in range(NCORES):
        cs = slice(c * COLS, (c + 1) * COLS)
        in_maps.append({
            "xt": XT,
            "wq": np.ascontiguousarray(Wq[:, cs]),
            "wk": np.ascontiguousarray(Wk[:, cs]),
            "wv": np.ascontiguousarray(Wv[:, cs]),
            "bq": np.ascontiguousarray(bq[cs].reshape(COLS, 1)),
            "bk": np.ascontiguousarray(bk[cs].reshape(COLS, 1)),
            "bv": np.ascontiguousarray(bv[cs].reshape(COLS, 1)),
            "wo": np.ascontiguousarray(Wo[cs, :]),
        })

    res = run_bass_kernel_spmd(
        nc, in_maps, core_ids=list(range(NCORES)),
        trace=_profile,
        trace_cores=(_trace_cores if _trace_cores is not None
                     else ([0] if _profile else None)),
    )

    ot = res.results[0]["ot"].astype(np.float64)
    for c in range(1, NCORES):
        ot += res.results[c]["ot"]
    out = (ot.T + bo).astype(np.float32).reshape(B, S, D)
    if _profile:
        kernel.last_exec_time_ns = res.exec_time_ns
        kernel.last_results = res
    return out



# revision 2
# speedup vs baseline: 1.0319x; 1.0319x over previous
"""Multi-head self-attention on 8 TRN2 NeuronCores, v3.

Sharding: (batch, head-quad). Core c owns batch c//4 and heads
4*(c%4)..4*(c%4)+3, i.e. a 256-column slice of Wq/Wk/Wv and the matching
256-row slice of Wo. The host sums the 4 partial outputs per batch (the
tensor-parallel all-reduce) and adds the output bias.

Tricks:
- X is augmented with a ones-row (a 1-partition SBUF memset) so the QKV
  biases AND the softmax-normalizer ones-column of V' fall out of the
  projection matmuls: Wv is augmented per-head with a unit column whose
  only nonzero is in the ones-row, so V' = [V_h | 1] per head.
- V' is projected directly into [token, col] layout (lhsT = X chunk), so
  attnV needs no transposes and no tile reassembly.
- Scores for two 512-query chunks land in one 2-bank PSUM tile and are
  exponentiated by a single 1024-wide activation.
- The attention inner loop is one flat (qp, h, kt) pipeline: the next
  score pair is emitted BEFORE the current attnV pair (PE never queues
  behind the exp dependency) and crosses h/qp boundaries; each query
  pair's output projection is drip-fed between the next segment's score
  pairs so the shared PSUM pool rotation alternates sc/po.
- The K projection runs d-outer across 8 PSUM banks so it pipelines with
  the X DMA; accumulating matmuls also warm the PE p-state.
- Softmax normalizer reciprocal rows are broadcast across partitions via
  a DRAM bounce (partition-stride-0 DMA read).

Precision: f16 activations/weights everywhere on the PE; PSUM f32;
partial outputs f32.
"""

import numpy as np

B, S, D, H, DK = 2, 2048, 1024, 16, 64
NCORES = 8
GPB = 4                    # head-groups per batch
HPC = H // GPB             # heads per core = 4
COLS = HPC * DK            # feature columns per core = 256
VCOLS = HPC * (DK + 1)     # V' columns incl per-head ones = 260
ND = D // 128              # 8 contraction chunks
NT = S // 512              # 4 token chunks (free dim 512)
NKT = S // 128             # 16 key tiles
NQP = S // 1024            # 2 query pairs
NE = D // 128              # 8 output-column chunks

_CACHE = {}


def _build_program():
    from contextlib import ExitStack

    import concourse.bacc as bacc
    import concourse.bass as bass
    import concourse.mybir as mybir
    import concourse.tile as tile

    f32 = mybir.dt.float32
    f32r = mybir.dt.float32r
    f16 = mybir.dt.float16
    Exp = mybir.ActivationFunctionType.Exp
    Mult = mybir.AluOpType.mult

    nc = bacc.Bacc("TRN2", target_bir_lowering=False, debug=False,
                   num_devices=NCORES)

    XT = nc.dram_tensor("xt", [D, S], f16, kind="ExternalInput").ap()
    WQ = nc.dram_tensor("wq", [D + 1, COLS], f16, kind="ExternalInput").ap()
    WK = nc.dram_tensor("wk", [D + 1, COLS], f16, kind="ExternalInput").ap()
    WV = nc.dram_tensor("wv", [D + 1, VCOLS], f16, kind="ExternalInput").ap()
    WO = nc.dram_tensor("wo", [COLS, D], f16, kind="ExternalInput").ap()
    OT = nc.dram_tensor("ot", [D, S], f16, kind="ExternalOutput").ap()

    with tile.TileContext(nc) as tc, ExitStack() as ctx:
        consts = ctx.enter_context(tc.tile_pool(name="consts", bufs=1))
        drp = ctx.enter_context(tc.tile_pool(name="drp", bufs=2, space="DRAM"))

        # ---- weights + X chunks ----
        xc = []
        for d in range(ND):
            t = consts.tile([128, S], f16, name=f"xc_{d}")
            nc.gpsimd.dma_start(t, XT[d * 128:(d + 1) * 128, :])
            xc.append(t)

        wq_c, wk_c, wv_c = [], [], []
        for d in range(ND):
            for nm, src, lst, w in (("wq", WQ, wq_c, COLS),
                                    ("wk", WK, wk_c, COLS),
                                    ("wv", WV, wv_c, VCOLS)):
                t = consts.tile([128, w], f16, name=f"{nm}_{d}")
                nc.sync.dma_start(t, src[d * 128:(d + 1) * 128, :])
                lst.append(t)
        wq_b = consts.tile([1, COLS], f16, name="wq_b")
        nc.sync.dma_start(wq_b, WQ[D:D + 1, :])
        wk_b = consts.tile([1, COLS], f16, name="wk_b")
        nc.sync.dma_start(wk_b, WK[D:D + 1, :])
        wv_b = consts.tile([1, VCOLS], f16, name="wv_b")
        nc.sync.dma_start(wv_b, WV[D:D + 1, :])
        wo_sb = []
        for cc in range(2):
            t = consts.tile([128, D], f16, name=f"wo_{cc}")
            nc.sync.dma_start(t, WO[cc * 128:(cc + 1) * 128, :])
            wo_sb.append(t)

        ones_row = consts.tile([1, S], f16, name="ones_row")
        nc.vector.memset(ones_row, 1.0)
        onecol_16 = consts.tile([1, DK], f16, name="onecol")
        nc.vector.memset(onecol_16, 1.0)

        KT = [consts.tile([128, S], f16, name=f"KT_{ct}") for ct in range(2)]
        QT = [consts.tile([128, S], f16, name=f"QT_{ct}") for ct in range(2)]
        VP = [consts.tile([128, VCOLS], f16, name=f"VP_{kt}")
              for kt in range(NKT)]

        # ---- phase 1: projections ----
        with tc.tile_pool(name="pk", bufs=8, space="PSUM") as pk:
            # K proj, d-outer so matmuls consume X chunks as they arrive
            kps = [pk.tile([128, 512], f32, tag="k", name=f"kps_{i}")
                   for i in range(8)]
            for d in range(ND + 1):
                for ct in range(2):
                    cs = slice(ct * 128, (ct + 1) * 128)
                    for tch in range(NT):
                        ts = slice(tch * 512, (tch + 1) * 512)
                        if d < ND:
                            lhsT, rhs = wk_c[d][:, cs], xc[d][:, ts]
                        else:
                            lhsT, rhs = wk_b[:, cs], ones_row[:, ts]
                        nc.tensor.matmul(kps[ct * NT + tch], lhsT=lhsT,
                                         rhs=rhs, start=(d == 0),
                                         stop=(d == ND))
            for ct in range(2):
                for tch in range(NT):
                    ts = slice(tch * 512, (tch + 1) * 512)
                    if (ct * NT + tch) % 2:
                        nc.scalar.copy(KT[ct][:, ts], kps[ct * NT + tch])
                    else:
                        nc.vector.tensor_copy(KT[ct][:, ts],
                                              kps[ct * NT + tch])

        with tc.tile_pool(name="pq", bufs=4, space="PSUM") as pq, \
                tc.tile_pool(name="pv", bufs=4, space="PSUM") as pv:
            # Q proj, tile-outer (X fully resident by now)
            for ct in range(2):
                cs = slice(ct * 128, (ct + 1) * 128)
                for tch in range(NT):
                    ts = slice(tch * 512, (tch + 1) * 512)
                    p = pq.tile([128, 512], f32, tag="q",
                                name=f"qps_{ct}_{tch}")
                    for d in range(ND + 1):
                        if d < ND:
                            lhsT, rhs = wq_c[d][:, cs], xc[d][:, ts]
                        else:
                            lhsT, rhs = wq_b[:, cs], ones_row[:, ts]
                        nc.tensor.matmul(p, lhsT=lhsT, rhs=rhs,
                                         start=(d == 0), stop=(d == ND))
                    if (ct * NT + tch) % 2:
                        nc.scalar.copy(QT[ct][:, ts], p)
                    else:
                        nc.vector.tensor_copy(QT[ct][:, ts], p)
            # V' proj directly in [token, vcol] layout
            for kt in range(NKT):
                ks = slice(kt * 128, (kt + 1) * 128)
                p = pv.tile([128, VCOLS], f32, tag="v", name=f"vps_{kt}")
                for d in range(ND + 1):
                    if d < ND:
                        lhsT, rhs = xc[d][:, ks], wv_c[d]
                    else:
                        lhsT, rhs = ones_row[:, ks], wv_b
                    nc.tensor.matmul(p, lhsT=lhsT, rhs=rhs,
                                     start=(d == 0), stop=(d == ND))
                if kt % 2:
                    nc.scalar.copy(VP[kt], p)
                else:
                    nc.vector.tensor_copy(VP[kt], p)

        # ---- phases 2+3: flat attention + outproj pipeline ----
        with tc.tile_pool(name="psc", bufs=2, space="PSUM") as psc, \
                tc.tile_pool(name="pat", bufs=2, space="PSUM") as pat, \
                tc.tile_pool(name="expp", bufs=3) as expp, \
                tc.tile_pool(name="attnp", bufs=4) as attnp, \
                tc.tile_pool(name="zp", bufs=4) as zp, \
                tc.tile_pool(name="zbp", bufs=2) as zbp, \
                tc.tile_pool(name="obp", bufs=4) as obp:

            segs = [(qp, h) for qp in range(NQP) for h in range(HPC)]
            attn_tiles = {}
            for qp in range(NQP):
                attn_tiles[qp] = [
                    attnp.tile([128, 1024], f16, tag="attn",
                               name=f"attn_{qp}_{ct}") for ct in range(2)]

            def sc_pair(qp, h, kt):
                ct, hs = h // 2, slice((h % 2) * 64, (h % 2) * 64 + 64)
                qs0 = slice(qp * 1024, qp * 1024 + 512)
                qs1 = slice(qp * 1024 + 512, (qp + 1) * 1024)
                ks = slice(kt * 128, (kt + 1) * 128)
                sc = psc.tile([128, 1024], f32, tag="s",
                              name=f"sc_{qp}_{h}_{kt}")
                nc.tensor.matmul(sc[:, 0:512], lhsT=KT[ct][hs, ks],
                                 rhs=QT[ct][hs, qs0], start=True, stop=True)
                nc.tensor.matmul(sc[:, 512:1024], lhsT=KT[ct][hs, ks],
                                 rhs=QT[ct][hs, qs1], start=True, stop=True)
                return sc

            def outproj_item(qp, e):
                es = slice(e * 128, (e + 1) * 128)
                po = psc.tile([128, 1024], f32, tag="s", name=f"po_{qp}_{e}")
                for qsl in (slice(0, 512), slice(512, 1024)):
                    for cc in range(2):
                        nc.tensor.matmul(po[:, qsl], lhsT=wo_sb[cc][:, es],
                                         rhs=attn_tiles[qp][cc][:, qsl],
                                         start=(cc == 0), stop=(cc == 1))
                ob = obp.tile([128, 1024], f16, tag="ob", name=f"ob_{qp}_{e}")
                if qp == NQP - 1 and e % 2:
                    # tail: ACT is idle, alternate whole-tile drains so two
                    # engines overlap and the PSUM slot frees at single-drain
                    # latency
                    nc.scalar.copy(ob, po)
                else:
                    nc.vector.tensor_copy(ob, po)
                qbase = qp * 1024
                nc.sync.dma_start(OT[es, qbase:qbase + 512], ob[:, 0:512])
                nc.gpsimd.dma_start(OT[es, qbase + 512:qbase + 1024],
                                    ob[:, 512:1024])

            pending_out = []          # deferred outproj (qp, e) items
            sc = sc_pair(0, 0, 0)
            for si, (qp, h) in enumerate(segs):
                ct, hs = h // 2, slice((h % 2) * 64, (h % 2) * 64 + 64)
                vs = slice(h * (DK + 1), (h + 1) * (DK + 1))
                patt = pat.tile([DK + 1, 1024], f32, tag="p",
                                name=f"patt_{qp}_{h}")
                for kt in range(NKT):
                    ex = expp.tile([128, 1024], f16, tag="e",
                                   name=f"ex_{qp}_{h}_{kt}")
                    nc.scalar.activation(ex, sc, Exp, scale=0.125)
                    # prefetch the next score pair (crossing h/qp bounds)
                    if kt + 1 < NKT:
                        sc = sc_pair(qp, h, kt + 1)
                    elif si + 1 < len(segs):
                        sc = sc_pair(segs[si + 1][0], segs[si + 1][1], 0)
                    # drip one deferred outproj chunk between score pairs
                    # (start late enough for the normalize chain to finish)
                    if pending_out and kt >= 4:
                        outproj_item(*pending_out.pop(0))
                    nc.tensor.matmul(patt[:, 0:512], lhsT=VP[kt][:, vs],
                                     rhs=ex[:, 0:512], start=(kt == 0),
                                     stop=(kt == NKT - 1))
                    nc.tensor.matmul(patt[:, 512:1024], lhsT=VP[kt][:, vs],
                                     rhs=ex[:, 512:1024], start=(kt == 0),
                                     stop=(kt == NKT - 1))
                # normalize head h: reciprocal of the ones-column row,
                # broadcast across 64 partitions, multiply
                zs = zp.tile([1, 1024], f32, tag="zs", name=f"zs_{qp}_{h}")
                nc.vector.tensor_copy(zs, patt[DK:DK + 1, :])
                zr = zp.tile([1, 1024], f32, tag="z", name=f"zr_{qp}_{h}")
                nc.vector.reciprocal_approx_fast(zr, zs)
                if si == len(segs) - 1:
                    # tail: no DRAM-bounce latency — drain attn rows, then
                    # broadcast 1/z via a PE outer product into free PSUM
                    araw = zbp.tile([DK, 1024], f32, tag="zb",
                                    name=f"araw_{qp}_{h}")
                    nc.vector.tensor_copy(araw, patt[0:DK, :])
                    zr16 = zp.tile([1, 1024], f16, tag="z16",
                                   name=f"zr16_{qp}_{h}")
                    nc.vector.tensor_copy(zr16, zr)
                    zbp_ps = psc.tile([128, 1024], f32, tag="s",
                                      name=f"zbps_{qp}_{h}")
                    nc.tensor.matmul(zbp_ps[0:DK, 0:512],
                                     lhsT=onecol_16[:, 0:DK],
                                     rhs=zr16[:, 0:512], start=True, stop=True)
                    nc.tensor.matmul(zbp_ps[0:DK, 512:1024],
                                     lhsT=onecol_16[:, 0:DK],
                                     rhs=zr16[:, 512:1024],
                                     start=True, stop=True)
                    nc.vector.tensor_tensor(attn_tiles[qp][ct][hs, :],
                                            araw, zbp_ps[0:DK, :], Mult)
                else:
                    scratch = drp.tile([1, 1024], f32, tag="scr",
                                       name=f"scr_{qp}_{h}")
                    nc.gpsimd.dma_start(scratch, zr)
                    zb = zbp.tile([DK, 1024], f32, tag="zb",
                                  name=f"zb_{qp}_{h}")
                    row = scratch[0:1, :]
                    nc.gpsimd.dma_start(
                        zb,
                        bass.AP(tensor=row.tensor, offset=row.offset,
                                ap=[[0, DK]] + row.ap[1:]))
                    nc.vector.tensor_tensor(attn_tiles[qp][ct][hs, :],
                                            patt[0:DK, :], zb, Mult)
                if h == HPC - 1:
                    pending_out.extend((qp, e) for e in range(NE))
            # flush any remaining outproj items (last query pair)
            for item in pending_out:
                outproj_item(*item)

    nc.compile()
    return nc


def _get_program():
    if "nc" not in _CACHE:
        _CACHE["nc"] = _build_program()
    return _CACHE["nc"]


def _install_ntff_hook():
    """Provide the antenv.axon_hooks shim this container's antenv lacks so
    run_bass_kernel_spmd(trace=True) can capture NTFF profiles."""
    import sys
    import types

    try:
        import antenv

        if hasattr(antenv, "axon_hooks"):
            return
        mod = types.ModuleType("antenv.axon_hooks")
        mod._hook = None
        mod.set_axon_ntff_profile_hook = lambda h: setattr(mod, "_hook", h)
        mod.get_axon_ntff_profile_hook = lambda: mod._hook
        sys.modules["antenv.axon_hooks"] = mod
        antenv.axon_hooks = mod
        from trn_agent_boot.trn_boot import _ntff_profile_via_ctypes

        mod.set_axon_ntff_profile_hook(
            _ntff_profile_via_ctypes("/opt/axon/libaxon_pjrt.so"))
    except Exception:
        pass


def kernel(X, Wq, bq, Wk, bk, Wv, bv, Wo, bo, _profile=False, _trace_cores=None):
    from concourse.bass_utils import run_bass_kernel_spmd

    if _profile:
        _install_ntff_hook()

    nc = _get_program()

    X = np.asarray(X, np.float32)
    Wq, Wk, Wv, Wo = (np.asarray(w, np.float32) for w in (Wq, Wk, Wv, Wo))
    bq, bk, bv, bo = (np.asarray(v, np.float32) for v in (bq, bk, bv, bo))

    in_maps = []
    for c in range(NCORES):
        b, g = c // GPB, c % GPB
        cs = slice(g * COLS, (g + 1) * COLS)
        wq_aug = np.vstack([Wq[:, cs], bq[cs][None, :]])
        wk_aug = np.vstack([Wk[:, cs], bk[cs][None, :]])
        wv_aug = np.zeros((D + 1, VCOLS), np.float32)
        for h in range(HPC):
            hc = slice(g * COLS + h * DK, g * COLS + (h + 1) * DK)
            wv_aug[:D, h * (DK + 1):h * (DK + 1) + DK] = Wv[:, hc]
            wv_aug[D, h * (DK + 1):h * (DK + 1) + DK] = bv[hc]
            wv_aug[D, h * (DK + 1) + DK] = 1.0
        in_maps.append({
            "xt": np.ascontiguousarray(X[b].T).astype(np.float16),
            "wq": wq_aug.astype(np.float16),
            "wk": wk_aug.astype(np.float16),
            "wv": wv_aug.astype(np.float16),
            "wo": np.ascontiguousarray(Wo[cs, :]).astype(np.float16),
        })

    res = run_bass_kernel_spmd(
        nc, in_maps, core_ids=list(range(NCORES)),
        trace=_profile,
        trace_cores=(_trace_cores if _trace_cores is not None
                     else ([0] if _profile else None)),
    )

    out = np.empty((B, S, D), np.float32)
    for b in range(B):
        ot = res.results[b * GPB]["ot"].astype(np.float64)
        for g in range(1, GPB):
            ot += res.results[b * GPB + g]["ot"]
        out[b] = (ot.T + bo).astype(np.float32)
    if _profile:
        kernel.last_exec_time_ns = res.exec_time_ns
        kernel.last_results = res
    return out
